# revision 7
# baseline (speedup 1.0000x reference)
"""Trainium2 Bass kernel for nn_NeuralDevice (segment_reduce).

Architecture (per reference.py):
  two "eyes": h = relu(x @ Wr + br)            [N=1M nodes, 64] -> [N, 128]
              segment-mean over idx (B=65536)  -> [B, 128]
              e = relu(mean @ Wc + bc)         -> [B, 128]
  brain:      z = [e0, e1]; out = relu(z@Wb1+bb1) @ Wb2 + bb2   -> [B, 128]

Distribution: shuffle-by-key. Each of the 8 cores owns a contiguous range of
B/8 = 8192 segments; the host routes every node (x row + idx) of each eye to
the core owning its segment.  On-device, each core:
  - streams its x^T (host-pretransposed, bf16, with a ones-row for the bias),
  - matmuls 128-node tiles against Wr_aug -> PSUM [128 nodes, 128 feat],
  - relu+bf16-casts into scatter-source layout (count column = constant 1.0),
  - dma_scatter_add's each node row (h[128] | 1.0 | pad) into one of 4
    zero-initialized DRAM accumulators [8192+pad, 256] bf16,
  - then finishes its 8192 segments: sum the 4 accumulators, transpose to
    feature-major, divide by max(count,1), apply the eye linears + brain MLP,
    and write out^T [128, 8192] f32.

dma_scatter_add loses updates when two descriptors in flight target the same
row (verified on HW), so indices within one call must be unique: the host
deals each core's nodes into rounds by rank-within-segment (round r holds each
segment's r-th occurrence), rounds are split into calls of <= CALL_MAX rows,
padded to a schedule shared by all cores (pad rows point at a trash row).
Calls are assigned round-robin to the 4 accumulators; calls on the same
accumulator serialize through Tile's WAW dependency (DMA-completion ordered),
calls on different accumulators touch disjoint memory.
"""

import numpy as np
import ml_dtypes

from concourse import bass, mybir
import concourse.bacc as bacc
import concourse.tile as tile
from concourse.bass_utils import run_bass_kernel_spmd
from concourse.masks import make_identity

BF16 = ml_dtypes.bfloat16

# problem sizes (hardcoded per spec)
B_FULL = 65536
N_FULL = 1048576
IN_NF = 64
R_OUT = 128
C_OUT = 128
BRAIN_H = 256
BRAIN_OUT = 128

CORES = 8
P = 128
CALL_MAX = 4096      # rows per dma_scatter_add call (descriptor carveout bound)
K_ACC = 4            # accumulator copies for scatter pipelining
CHUNK = 8192         # x^T columns loaded per DMA
ACC_COLS = 256       # bf16: h[0:128], count at 128, junk 129:256
DMA_SCRATCH = 32768


# ----------------------------------------------------------------- planning

def _rank_within(seg):
    """rank[i] = number of j<i (in given order) with seg[j]==seg[i]."""
    n = len(seg)
    if n == 0:
        return np.zeros(0, np.int64)
    order = np.argsort(seg, kind="stable")
    ss = seg[order]
    newgrp = np.r_[True, ss[1:] != ss[:-1]]
    starts = np.flatnonzero(newgrp)
    gid = np.cumsum(newgrp) - 1
    rank_sorted = np.arange(n) - starts[gid]
    rank = np.empty(n, np.int64)
    rank[order] = rank_sorted
    return rank


def _plan(idx_eyes, segs_per_core):
    """Build the global call schedule + per-(eye, core) node placement.

    Returns (calls, total, placements) where
      calls = list of (size,) call sizes (each a multiple of 128, <= CALL_MAX)
      total = sum of sizes
      placements[e][c] = (nodes, seg_rel, pos) with pos = stream slot per node
    """
    n_eyes = len(idx_eyes)
    per_ce = {}
    hists = []
    for e in range(n_eyes):
        idx = idx_eyes[e]
        owner = idx // segs_per_core
        for c in range(CORES):
            nodes = np.flatnonzero(owner == c)
            seg_rel = (idx[nodes] - c * segs_per_core).astype(np.int64)
            rank = _rank_within(seg_rel)
            per_ce[(e, c)] = (nodes, seg_rel, rank)
            hists.append(np.bincount(rank))
    r_max = max(len(h) for h in hists)
    round_size = np.zeros(r_max, np.int64)
    for h in hists:
        round_size[: len(h)] = np.maximum(round_size[: len(h)], h)
    round_size = ((round_size + 127) // 128) * 128

    calls = []          # (round, offset_in_round, size)
    for r in range(r_max):
        off = 0
        while off < round_size[r]:
            sz = min(CALL_MAX, round_size[r] - off)
            calls.append((r, off, int(sz)))
            off += sz
    bases = np.cumsum([0] + [sz for (_, _, sz) in calls])
    total = int(bases[-1])

    placements = {}
    for (e, c), (nodes, seg_rel, rank) in per_ce.items():
        order = np.argsort(rank, kind="stable")
        rs = np.searchsorted(rank[order], np.arange(r_max + 1))
        pos = np.empty(len(nodes), np.int64)
        for ci, (r, off, sz) in enumerate(calls):
            lo = rs[r] + off
            hi = min(rs[r] + off + sz, rs[r + 1])
            if hi <= lo:
                continue
            take = order[lo:hi]
            pos[take] = bases[ci] + np.arange(hi - lo)
        placements[(e, c)] = (nodes, seg_rel, pos)
    return [sz for (_, _, sz) in calls], total, placements


def _wrap_idx(idxvals, calls):
    """Per-call [16, sz/16] wrap layout, concatenated, tiled to 128 partitions."""
    parts = []
    base = 0
    for sz in calls:
        blk = idxvals[base : base + sz].reshape(-1, 16).T
        parts.append(blk)
        base += sz
    w = np.ascontiguousarray(np.hstack(parts).astype(np.int16))
    return np.tile(w, (8, 1))


# ------------------------------------------------------------ program build

_NC_CACHE = {}


def _build_nc(total, calls, segs):
    key = (total, tuple(calls), segs)
    if key in _NC_CACHE:
        return _NC_CACHE[key]

    trash = segs
    acc_rows = segs + 128
    f32 = mybir.dt.float32
    bf16 = mybir.dt.bfloat16
    i16 = mybir.dt.int16
    relu = mybir.ActivationFunctionType.Relu

    nc = bacc.Bacc(
        "TRN2",
        target_bir_lowering=False,
        debug=False,
        dynamic_dma_scratch_size=DMA_SCRATCH,
    )

    xT_d = [
        nc.dram_tensor(f"x{e}T", [IN_NF + 1, total], bf16, kind="ExternalInput")
        for e in range(2)
    ]
    idx_d = [
        nc.dram_tensor(f"idx{e}", [128, total // 16], i16, kind="ExternalInput")
        for e in range(2)
    ]
    wr_d = [nc.dram_tensor(f"wr{e}", [IN_NF + 1, R_OUT], bf16, kind="ExternalInput") for e in range(2)]
    wc_d = [nc.dram_tensor(f"wc{e}", [R_OUT, C_OUT], bf16, kind="ExternalInput") for e in range(2)]
    bc_d = [nc.dram_tensor(f"bc{e}", [C_OUT, 1], f32, kind="ExternalInput") for e in range(2)]
    wb1lo_d = nc.dram_tensor("wb1lo", [128, BRAIN_H], bf16, kind="ExternalInput")
    wb1hi_d = nc.dram_tensor("wb1hi", [128, BRAIN_H], bf16, kind="ExternalInput")
    bb1a_d = nc.dram_tensor("bb1a", [128, 1], f32, kind="ExternalInput")
    bb1b_d = nc.dram_tensor("bb1b", [128, 1], f32, kind="ExternalInput")
    wb2lo_d = nc.dram_tensor("wb2lo", [128, BRAIN_OUT], bf16, kind="ExternalInput")
    wb2hi_d = nc.dram_tensor("wb2hi", [128, BRAIN_OUT], bf16, kind="ExternalInput")
    bb2_d = nc.dram_tensor("bb2", [BRAIN_OUT, 1], f32, kind="ExternalInput")

    acc_d = [
        [
            nc.dram_tensor(f"acc{e}_{k}", [acc_rows, ACC_COLS], bf16, kind="ExternalOutput")
            for k in range(K_ACC)
        ]
        for e in range(2)
    ]
    outT_d = nc.dram_tensor("outT", [128, segs], f32, kind="ExternalOutput")

    bases = np.cumsum([0] + list(calls))
    n_calls = len(calls)
    # block -> (call index, slot within call)
    call_of_block = []
    slot_of_block = []
    for ci, sz in enumerate(calls):
        for s in range(sz // 128):
            call_of_block.append(ci)
            slot_of_block.append(s)
    nblocks = total // 128
    assert len(call_of_block) == nblocks

    with tile.TileContext(nc) as tc:
        with tc.tile_pool(name="consts", bufs=1) as cp:
            ident = cp.tile([128, 128], bf16)
            make_identity(nc, ident[:])
            wr_t = [cp.tile([IN_NF + 1, R_OUT], bf16, tag=f"wr{e}", name=f"wr{e}t") for e in range(2)]
            wc_t = [cp.tile([R_OUT, C_OUT], bf16, tag=f"wc{e}", name=f"wc{e}t") for e in range(2)]
            bc_t = [cp.tile([C_OUT, 1], f32, tag=f"bc{e}", name=f"bc{e}t") for e in range(2)]
            wb1lo_t = cp.tile([128, BRAIN_H], bf16, tag="wb1lo")
            wb1hi_t = cp.tile([128, BRAIN_H], bf16, tag="wb1hi")
            bb1a_t = cp.tile([128, 1], f32, tag="bb1a")
            bb1b_t = cp.tile([128, 1], f32, tag="bb1b")
            wb2lo_t = cp.tile([128, BRAIN_OUT], bf16, tag="wb2lo")
            wb2hi_t = cp.tile([128, BRAIN_OUT], bf16, tag="wb2hi")
            bb2_t = cp.tile([BRAIN_OUT, 1], f32, tag="bb2")
            for e in range(2):
                nc.sync.dma_start(out=wr_t[e][:], in_=wr_d[e][:])
                nc.sync.dma_start(out=wc_t[e][:], in_=wc_d[e][:])
                nc.sync.dma_start(out=bc_t[e][:], in_=bc_d[e][:])
            nc.sync.dma_start(out=wb1lo_t[:], in_=wb1lo_d[:])
            nc.sync.dma_start(out=wb1hi_t[:], in_=wb1hi_d[:])
            nc.sync.dma_start(out=bb1a_t[:], in_=bb1a_d[:])
            nc.sync.dma_start(out=bb1b_t[:], in_=bb1b_d[:])
            nc.sync.dma_start(out=wb2lo_t[:], in_=wb2lo_d[:])
            nc.sync.dma_start(out=wb2hi_t[:], in_=wb2hi_d[:])
            nc.sync.dma_start(out=bb2_t[:], in_=bb2_d[:])

            # ------------------------------------------------ main phase
            with (
                tc.tile_pool(name="xch", bufs=2) as xp,
                tc.tile_pool(name="src", bufs=5) as sp,
                tc.tile_pool(name="sidx", bufs=6) as ixp,
                tc.tile_pool(name="mm", bufs=6, space="PSUM") as mp,
            ):
                for e in range(2):
                    src_tiles = [None] * n_calls
                    blocks_done = [0] * n_calls
                    alt = 0
                    for cbase in range(0, total, CHUNK):
                        csize = min(CHUNK, total - cbase)
                        xt = xp.tile([IN_NF + 1, CHUNK], bf16, tag="xch", name=f"xch_e{e}_{cbase}")
                        nc.sync.dma_start(
                            out=xt[:, :csize], in_=xT_d[e][:, cbase : cbase + csize]
                        )
                        for g0 in range(0, csize, 512):
                            gsz = min(512, csize - g0)
                            nb = gsz // 128
                            ps = mp.tile([128, 4, 128], f32, tag="mm", name=f"mm_e{e}_{cbase}_{g0}")
                            for j in range(nb):
                                nc.tensor.matmul(
                                    out=ps[:, j, :],
                                    lhsT=xt[0 : IN_NF + 1, g0 + j * 128 : g0 + (j + 1) * 128],
                                    rhs=wr_t[e][:],
                                    start=True,
                                    stop=True,
                                )
                            # relu + cast into scatter-src slots, span per call
                            b0 = (cbase + g0) // 128
                            j0 = 0
                            while j0 < nb:
                                ci = call_of_block[b0 + j0]
                                j1 = j0
                                while j1 < nb and call_of_block[b0 + j1] == ci:
                                    j1 += 1
                                nblk = j1 - j0
                                if src_tiles[ci] is None:
                                    sz = calls[ci]
                                    st = sp.tile([128, CALL_MAX // 128, ACC_COLS], bf16, tag="src", name=f"src_e{e}_c{ci}")
                                    src_tiles[ci] = st
                                    nc.vector.memset(st[:, : sz // 128, 128:129], 1.0)
                                    nc.vector.memset(st[:, : sz // 128, 129:256], 0.0)
                                st = src_tiles[ci]
                                s0 = slot_of_block[b0 + j0]
                                if alt % 2 == 0:
                                    nc.scalar.activation(
                                        out=st[:, s0 : s0 + nblk, 0:128],
                                        in_=ps[:, j0 : j0 + nblk, :],
                                        func=relu,
                                    )
                                else:
                                    nc.vector.tensor_scalar_max(
                                        st[:, s0 : s0 + nblk, 0:128],
                                        ps[:, j0 : j0 + nblk, :],
                                        0.0,
                                    )
                                alt += 1
                                blocks_done[ci] += nblk
                                if blocks_done[ci] == calls[ci] // 128:
                                    sz = calls[ci]
                                    ixt = ixp.tile([128, CALL_MAX // 16], i16, tag="sidx", name=f"idx_e{e}_c{ci}")
                                    nc.sync.dma_start(
                                        out=ixt[:, : sz // 16],
                                        in_=idx_d[e][:, bases[ci] // 16 : (bases[ci] + sz) // 16],
                                    )
                                    nc.gpsimd.dma_scatter_add(
                                        acc_d[e][ci % K_ACC][:],
                                        st[:, : sz // 128, :],
                                        ixt[:, : sz // 16],
                                        sz,
                                        sz,
                                        ACC_COLS,
                                    )
                                    src_tiles[ci] = False  # done
                                j0 = j1

            # ------------------------------------------------ finish phase
            with (
                tc.tile_pool(name="fin", bufs=3) as fp,
                tc.tile_pool(name="pm", bufs=2, space="PSUM") as pm,
                tc.tile_pool(name="pe", bufs=2, space="PSUM") as pe,
                tc.tile_pool(name="ph", bufs=2, space="PSUM") as ph,
                tc.tile_pool(name="py", bufs=2, space="PSUM") as py,
            ):
                for t in range(segs // 512):
                    r0 = t * 512
                    eT = [None, None]
                    for e in range(2):
                        at = []
                        for k in range(K_ACC):
                            a = fp.tile([128, 4, ACC_COLS], bf16, tag=f"acck{k}", name=f"acck_{t}_{e}_{k}")
                            src_ap = acc_d[e][k][r0 : r0 + 512, :].rearrange(
                                "(s p) d -> p s d", p=128
                            )
                            nc.sync.dma_start(out=a[:], in_=src_ap)
                            at.append(a)
                        s01 = fp.tile([128, 4, ACC_COLS], bf16, tag="s01")
                        s23 = fp.tile([128, 4, ACC_COLS], bf16, tag="s23")
                        sm = fp.tile([128, 4, ACC_COLS], bf16, tag="sm")
                        nc.vector.tensor_add(s01[:], at[0][:], at[1][:])
                        nc.vector.tensor_add(s23[:], at[2][:], at[3][:])
                        nc.vector.tensor_add(sm[:], s01[:], s23[:])
                        # mean = sum / max(cnt, 1), applied seg-major (per-
                        # partition scalar), then transpose to feature-major
                        cnt4 = fp.tile([128, 4, 1], f32, tag="cnt4")
                        rcp4 = fp.tile([128, 4, 1], f32, tag="rcp4")
                        nc.vector.tensor_scalar_max(cnt4[:], sm[:, :, 128:129], 1.0)
                        nc.vector.reciprocal(rcp4[:], cnt4[:])
                        meanp = fp.tile([128, 4, 128], bf16, tag="meanp")
                        psm = pm.tile([128, 512], bf16, tag="pm")
                        for k4 in range(4):
                            nc.vector.tensor_scalar_mul(
                                meanp[:, k4, :], sm[:, k4, 0:128], rcp4[:, k4, 0:1]
                            )
                            nc.tensor.transpose(
                                out=psm[:, k4 * 128 : (k4 + 1) * 128],
                                in_=meanp[:, k4, :],
                                identity=ident[:],
                            )
                        meanT = fp.tile([128, 512], bf16, tag="meanT")
                        nc.vector.tensor_copy(meanT[:], psm[:])
                        pse = pe.tile([128, 512], f32, tag="pe")
                        nc.tensor.matmul(
                            out=pse[:], lhsT=wc_t[e][:], rhs=meanT[:], start=True, stop=True
                        )
                        et = fp.tile([128, 512], bf16, tag=f"eT{e}")
                        nc.scalar.activation(
                            out=et[:], in_=pse[:], func=relu, bias=bc_t[e][:, 0:1]
                        )
                        eT[e] = et
                    psh_a = ph.tile([128, 512], f32, tag="ph")
                    nc.tensor.matmul(out=psh_a[:], lhsT=wb1lo_t[:, 0:128], rhs=eT[0][:], start=True, stop=False)
                    nc.tensor.matmul(out=psh_a[:], lhsT=wb1hi_t[:, 0:128], rhs=eT[1][:], start=False, stop=True)
                    hTa = fp.tile([128, 512], bf16, tag="hTa")
                    nc.scalar.activation(out=hTa[:], in_=psh_a[:], func=relu, bias=bb1a_t[:, 0:1])
                    psh_b = ph.tile([128, 512], f32, tag="ph")
                    nc.tensor.matmul(out=psh_b[:], lhsT=wb1lo_t[:, 128:256], rhs=eT[0][:], start=True, stop=False)
                    nc.tensor.matmul(out=psh_b[:], lhsT=wb1hi_t[:, 128:256], rhs=eT[1][:], start=False, stop=True)
                    hTb = fp.tile([128, 512], bf16, tag="hTb")
                    nc.scalar.activation(out=hTb[:], in_=psh_b[:], func=relu, bias=bb1b_t[:, 0:1])
                    psy = py.tile([128, 512], f32, tag="py")
                    nc.tensor.matmul(out=psy[:], lhsT=wb2lo_t[:], rhs=hTa[:], start=True, stop=False)
                    nc.tensor.matmul(out=psy[:], lhsT=wb2hi_t[:], rhs=hTb[:], start=False, stop=True)
                    ys = fp.tile([128, 512], f32, tag="ys")
                    nc.vector.tensor_scalar_add(ys[:], psy[:], bb2_t[:, 0:1])
                    nc.sync.dma_start(out=outT_d[:, r0 : r0 + 512], in_=ys[:])

    nc.compile()
    _NC_CACHE[key] = nc
    return nc


# ------------------------------------------------------------------ driver

def _prepare(inputs, b_full):
    segs = b_full // CORES
    x = [np.asarray(inputs["x0"]), np.asarray(inputs["x1"])]
    idx = [
        np.asarray(inputs["idx0"]).astype(np.int64),
        np.asarray(inputs["idx1"]).astype(np.int64),
    ]
    calls, total, placements = _plan(idx, segs)

    shared = {}
    for e in range(2):
        wr = np.asarray(inputs[f"Wr{e}"])
        br = np.asarray(inputs[f"br{e}"])
        shared[f"wr{e}"] = np.vstack([wr, br[None, :]]).astype(BF16)
        shared[f"wc{e}"] = np.asarray(inputs[f"Wc{e}"]).astype(BF16)
        shared[f"bc{e}"] = np.asarray(inputs[f"bc{e}"]).astype(np.float32).reshape(-1, 1)
    wb1 = np.asarray(inputs["Wb1"])
    bb1 = np.asarray(inputs["bb1"])
    wb2 = np.asarray(inputs["Wb2"])
    bb2 = np.asarray(inputs["bb2"])
    shared["wb1lo"] = wb1[0:128].astype(BF16)
    shared["wb1hi"] = wb1[128:256].astype(BF16)
    shared["bb1a"] = bb1[0:128].astype(np.float32).reshape(-1, 1)
    shared["bb1b"] = bb1[128:256].astype(np.float32).reshape(-1, 1)
    shared["wb2lo"] = wb2[0:128].astype(BF16)
    shared["wb2hi"] = wb2[128:256].astype(BF16)
    shared["bb2"] = bb2.astype(np.float32).reshape(-1, 1)

    in_maps = []
    for c in range(CORES):
        m = dict(shared)
        for e in range(2):
            nodes, seg_rel, pos = placements[(e, c)]
            arr = np.zeros((total, IN_NF + 1), np.float32)
            arr[pos, :IN_NF] = x[e][nodes]
            arr[pos, IN_NF] = 1.0
            m[f"x{e}T"] = np.ascontiguousarray(arr.T).astype(BF16)
            idxvals = np.full(total, segs, np.int32)  # trash row
            idxvals[pos] = seg_rel
            m[f"idx{e}"] = _wrap_idx(idxvals, calls)
        in_maps.append(m)
    return calls, total, segs, in_maps


def _axon_reset():
    try:
        import ctypes

        lib = ctypes.CDLL("/opt/axon/libaxon_pjrt.so")
        lib.axon_reset.restype = ctypes.c_int
        lib.axon_reset()
    except Exception:
        pass


def _run(inputs, trace=False, trace_kwargs=None):
    calls, total, segs, in_maps = _prepare(inputs, B_FULL)
    nc = _build_nc(total, calls, segs)
    try:
        res = run_bass_kernel_spmd(
            nc,
            in_maps,
            list(range(CORES)),
            trace=trace,
            **(trace_kwargs or {}),
        )
    except Exception as e:
        if "UNRECOVERABLE" not in str(e) and "UNAVAILABLE" not in str(e):
            raise
        _axon_reset()
        res = run_bass_kernel_spmd(
            nc,
            in_maps,
            list(range(CORES)),
            trace=trace,
            **(trace_kwargs or {}),
        )
    out = np.concatenate([res.results[c]["outT"].T for c in range(CORES)], axis=0)
    return out.astype(np.float32), res


def kernel(**inputs):
    return _run(inputs)[0]
